# revision 6
# baseline (speedup 1.0000x reference)
"""Trainium2 Bass kernel for nn_DAHead_Channel (conv3x3+BN+ReLU -> channel attention -> conv3x3+BN+ReLU).

Data-parallel over batch across 8 NeuronCores (1 batch element each).
Training-mode BatchNorm needs global (N,H,W) stats -> two tiny AllReduces
([128,2] and [128,8] f32) between local compute phases.

Hardcoded problem shape: x [8,512,64,64] f32, w1 [128,512,3,3], w2 [512,128,3,3].
"""

import numpy as np

import concourse.bacc as bacc
import concourse.mybir as mybir
import concourse.tile as tile
from concourse import bass_utils

N_CORES = 8
B, CIN, H, W = 8, 512, 64, 64
CMID = 128
HP, WP = H + 2, W + 2          # zero-padded spatial dims
PIX = H * W                    # 4096
PPAD = HP * WP                 # 4356
NT = 8                         # HW tiles of 512 pixels (8 rows of 64)
NC1 = CIN // 128               # 4 input-channel chunks for conv1
NC2 = CIN // 128               # 4 output-channel chunks for conv2
NK = PIX // 128                # 32 transpose chunks for attention
EPS = 1e-5
NHW = B * PIX                  # BN count = 32768
F32 = mybir.dt.float32
BF16 = mybir.dt.bfloat16

CONV_BF16 = False              # compute convs in bf16 (f32 accumulation)
CONV_DT = BF16 if CONV_BF16 else F32


def _emit(nc):
    xin = nc.dram_tensor("xp", [CIN, PPAD], CONV_DT, kind="ExternalInput")
    w1in = nc.dram_tensor("w1t", [128, NC1 * 9 * 128], CONV_DT, kind="ExternalInput")
    w2in = nc.dram_tensor("w2t", [128, 9 * NC2 * 128], CONV_DT, kind="ExternalInput")
    g1in = nc.dram_tensor("g1b1", [128, 2], F32, kind="ExternalInput")
    g2in = nc.dram_tensor("g2b2", [128, 8], F32, kind="ExternalInput")
    btin = nc.dram_tensor("betav", [128, 1], F32, kind="ExternalInput")
    idin = nc.dram_tensor("ident", [128, 128], F32, kind="ExternalInput")
    out = nc.dram_tensor("out", [CIN, PIX], F32, kind="ExternalOutput")

    AF = mybir.ActivationFunctionType
    ALU = mybir.AluOpType
    AX = mybir.AxisListType

    with tile.TileContext(nc) as tc:
        with (
            tc.tile_pool(name="persist", bufs=1) as P,
            tc.tile_pool(name="rot", bufs=4) as R,
            tc.tile_pool(name="ps", bufs=8, space="PSUM") as PS,
            tc.tile_pool(name="dram", bufs=1, space="DRAM") as DR,
        ):
            # ---------------- persistent SBUF ----------------
            w2sb = P.tile([128, 9 * NC2 * 128], CONV_DT, tag="w2")
            h1 = P.tile([128, PIX], F32, tag="h1")
            xf = P.tile([128, PIX], F32, tag="xf")
            c2in = P.tile([128, PPAD], CONV_DT, tag="c2in")
            ident = P.tile([128, 128], F32, tag="ident")
            g1b1 = P.tile([128, 2], F32, tag="g1b1")
            g2b2 = P.tile([128, 8], F32, tag="g2b2")
            betav = P.tile([128, 1], F32, tag="betav")
            s1 = P.tile([128, NT], F32, tag="s1")        # per-tile rowsums conv1
            q1 = P.tile([128, NT], F32, tag="q1")        # per-tile rowsumsq conv1
            st1 = P.tile([128, 2], F32, tag="st1")       # packed local stats 1
            st1g = P.tile([128, 2], F32, tag="st1g")     # global stats 1
            s2 = P.tile([128, NC2 * NT], F32, tag="s2")
            q2 = P.tile([128, NC2 * NT], F32, tag="q2")
            st2 = P.tile([128, 8], F32, tag="st2")       # cols 0-3 sum, 4-7 sumsq
            st2g = P.tile([128, 8], F32, tag="st2g")
            co1 = P.tile([128, 10], F32, tag="co1")      # BN1 coef scratch
            co2 = P.tile([128, 32], F32, tag="co2")      # BN2 coef scratch (4-wide)
            att_e = P.tile([128, 128], F32, tag="att_e")
            attw = P.tile([128, 128], F32, tag="attw")
            attwT = P.tile([128, 128], F32, tag="attwT")
            asml = P.tile([128, 4], F32, tag="asml")     # softmax min/sum/recip

            nc.sync.dma_start(out=ident[:], in_=idin.ap()[:])
            nc.sync.dma_start(out=g1b1[:], in_=g1in.ap()[:])
            nc.sync.dma_start(out=g2b2[:], in_=g2in.ap()[:])
            nc.sync.dma_start(out=betav[:], in_=btin.ap()[:])
            nc.gpsimd.memset(c2in[:], 0.0)

            ar1i = DR.tile([128, 2], F32, tag="ar1i")
            ar1o = DR.tile([128, 2], F32, tag="ar1o")
            ar2i = DR.tile([128, 8], F32, tag="ar2i")
            ar2o = DR.tile([128, 8], F32, tag="ar2o")

            # ---------------- phase 1: conv1 + attention ----------------
            with tc.tile_pool(name="phase1", bufs=1) as P1:
                w1sb = P1.tile([128, NC1 * 9 * 128], CONV_DT, tag="w1")
                nc.sync.dma_start(out=w1sb[:], in_=w1in.ap()[:])
                xsb = []
                for c in range(NC1):
                    t = P1.tile([128, PPAD], CONV_DT, tag=f"x{c}", name=f"xsb{c}")
                    nc.sync.dma_start(
                        out=t[:], in_=xin.ap()[c * 128 : (c + 1) * 128, :]
                    )
                    xsb.append(t)
                # conv2 weights can land any time before conv2
                nc.sync.dma_start(out=w2sb[:], in_=w2in.ap()[:])

                ps1 = [
                    PS.tile([128, 512], F32, tag="ps", name=f"ps1_{t}")
                    for t in range(NT)
                ]
                for c in range(NC1):
                    xv = xsb[c][:].rearrange("p (h w) -> p h w", w=WP)
                    for o in range(9):
                        ky, kx = o // 3, o % 3
                        lhsT = w1sb[:, (c * 9 + o) * 128 : (c * 9 + o + 1) * 128]
                        for t in range(NT):
                            nc.tensor.matmul(
                                ps1[t][:],
                                lhsT,
                                xv[:, 8 * t + ky : 8 * t + ky + 8, kx : kx + W],
                                start=(c == 0 and o == 0),
                                stop=(c == NC1 - 1 and o == 8),
                                skip_group_check=True,
                            )
                # drain: copy to h1 (+rowsum on ACT), square (+rowsumsq on DVE)
                for t in range(NT):
                    sc = R.tile([128, 512], F32, tag="scr", name=f"scr1_{t}")
                    nc.scalar.activation(
                        out=h1[:, 512 * t : 512 * (t + 1)],
                        in_=ps1[t][:],
                        func=AF.Copy,
                        accum_out=s1[:, t : t + 1],
                    )
                    nc.vector.scalar_tensor_tensor(
                        out=sc[:],
                        in0=ps1[t][:],
                        scalar=1.0,
                        in1=h1[:, 512 * t : 512 * (t + 1)],
                        op0=ALU.mult,
                        op1=ALU.mult,
                        accum_out=q1[:, t : t + 1],
                    )

                # local stats -> AllReduce
                nc.vector.reduce_sum(st1[:, 0:1], s1[:], axis=AX.X)
                nc.vector.reduce_sum(st1[:, 1:2], q1[:], axis=AX.X)
                nc.sync.dma_start(out=ar1i[:], in_=st1[:])
                nc.gpsimd.collective_compute(
                    "AllReduce",
                    ALU.add,
                    replica_groups=[list(range(N_CORES))],
                    ins=[ar1i.opt()],
                    outs=[ar1o.opt()],
                )
                nc.sync.dma_start(out=st1g[:], in_=ar1o[:])

                # BN1 coefficients: a = gamma*rsqrt(var+eps), b = beta - mean*a
                mean, ex2, m2, var = co1[:, 0:1], co1[:, 1:2], co1[:, 2:3], co1[:, 3:4]
                sv, rsq, a1 = co1[:, 4:5], co1[:, 5:6], co1[:, 6:7]
                ma, b1 = co1[:, 7:8], co1[:, 8:9]
                inv_n = 1.0 / NHW
                nc.scalar.mul(mean, st1g[:, 0:1], inv_n)
                nc.scalar.mul(ex2, st1g[:, 1:2], inv_n)
                nc.scalar.square(m2, mean)
                nc.vector.scalar_tensor_tensor(
                    out=var, in0=ex2, scalar=EPS, in1=m2,
                    op0=ALU.add, op1=ALU.subtract,
                )
                nc.scalar.activation(sv, var, AF.Sqrt)
                nc.vector.reciprocal(rsq, sv)
                nc.vector.tensor_mul(a1, g1b1[:, 0:1], rsq)
                nc.vector.tensor_mul(ma, mean, a1)
                nc.vector.tensor_sub(b1, g1b1[:, 1:2], ma)

                # BN1 apply + ReLU
                for t in range(NT):
                    nc.scalar.activation(
                        out=xf[:, 512 * t : 512 * (t + 1)],
                        in_=h1[:, 512 * t : 512 * (t + 1)],
                        func=AF.Relu,
                        bias=b1,
                        scale=a1,
                    )

                # ---------------- attention ----------------
                att = PS.tile([128, 512], F32, tag="ps", name="att")
                for k in range(NK):
                    pst = PS.tile([128, 512], F32, tag="ps", name=f"pst{k}")
                    xfT = P1.tile([128, 128], F32, tag="xfT", bufs=3, name=f"xfT{k}")
                    nc.tensor.transpose(
                        pst[:, 0:128], xf[:, 128 * k : 128 * (k + 1)], ident[:]
                    )
                    nc.vector.tensor_copy(xfT[:], pst[:, 0:128])
                    nc.tensor.matmul(
                        att[:, 0:128],
                        xfT[:],
                        xfT[:],
                        start=(k == 0),
                        stop=(k == NK - 1),
                        skip_group_check=True,
                    )

                # softmax of (rowmax - att) over rows == exp(rowmin - att)/sum
                amin, asum, arcp = asml[:, 0:1], asml[:, 1:2], asml[:, 2:3]
                nc.vector.tensor_reduce(
                    out=amin, in_=att[:, 0:128], op=ALU.min, axis=AX.X
                )
                nc.scalar.activation(
                    out=att_e[:],
                    in_=att[:, 0:128],
                    func=AF.Exp,
                    bias=amin,
                    scale=-1.0,
                    accum_out=asum,
                )
                nc.vector.reciprocal(arcp, asum)
                nc.vector.tensor_scalar_mul(attw[:], att_e[:], arcp)
                pat = PS.tile([128, 512], F32, tag="ps", name="pat")
                nc.tensor.transpose(pat[:, 0:128], attw[:], ident[:])
                nc.vector.tensor_copy(attwT[:], pat[:, 0:128])

                # out = attw @ xf ; c2in = beta*out + xf (into padded interior)
                c2v = c2in[:].rearrange("p (h w) -> p h w", w=WP)
                for t in range(NT):
                    po = PS.tile([128, 512], F32, tag="ps", name=f"po{t}")
                    nc.tensor.matmul(
                        po[:], attwT[:], xf[:, 512 * t : 512 * (t + 1)],
                        start=True, stop=True, skip_group_check=True,
                    )
                    nc.vector.scalar_tensor_tensor(
                        out=c2v[:, 1 + 8 * t : 9 + 8 * t, 1 : 1 + W],
                        in0=po[:],
                        scalar=betav[:],
                        in1=xf[:, 512 * t : 512 * (t + 1)],
                        op0=ALU.mult,
                        op1=ALU.add,
                    )

            # ---------------- phase 2: conv2 ----------------
            with tc.tile_pool(name="phase2", bufs=1) as P2:
                h2 = [
                    P2.tile([128, PIX], F32, tag=f"h2_{cc}", name=f"h2_{cc}")
                    for cc in range(NC2)
                ]
                c2vv = c2in[:].rearrange("p (h w) -> p h w", w=WP)
                for cc in range(NC2):
                    ps2 = [
                        PS.tile([128, 512], F32, tag="ps", name=f"ps2_{cc}_{t}")
                        for t in range(NT)
                    ]
                    for o in range(9):
                        ky, kx = o // 3, o % 3
                        lhsT = w2sb[:, (o * NC2 + cc) * 128 : (o * NC2 + cc + 1) * 128]
                        for t in range(NT):
                            nc.tensor.matmul(
                                ps2[t][:],
                                lhsT,
                                c2vv[:, 8 * t + ky : 8 * t + ky + 8, kx : kx + W],
                                start=(o == 0),
                                stop=(o == 8),
                                skip_group_check=True,
                            )
                    for t in range(NT):
                        sc = R.tile([128, 512], F32, tag="scr", name=f"scr2_{cc}_{t}")
                        nc.scalar.activation(
                            out=h2[cc][:, 512 * t : 512 * (t + 1)],
                            in_=ps2[t][:],
                            func=AF.Copy,
                            accum_out=s2[:, cc * NT + t : cc * NT + t + 1],
                        )
                        nc.vector.scalar_tensor_tensor(
                            out=sc[:],
                            in0=ps2[t][:],
                            scalar=1.0,
                            in1=h2[cc][:, 512 * t : 512 * (t + 1)],
                            op0=ALU.mult,
                            op1=ALU.mult,
                            accum_out=q2[:, cc * NT + t : cc * NT + t + 1],
                        )

                for cc in range(NC2):
                    nc.vector.reduce_sum(
                        st2[:, cc : cc + 1], s2[:, cc * NT : (cc + 1) * NT], axis=AX.X
                    )
                    nc.vector.reduce_sum(
                        st2[:, 4 + cc : 5 + cc],
                        q2[:, cc * NT : (cc + 1) * NT],
                        axis=AX.X,
                    )
                nc.sync.dma_start(out=ar2i[:], in_=st2[:])
                nc.gpsimd.collective_compute(
                    "AllReduce",
                    ALU.add,
                    replica_groups=[list(range(N_CORES))],
                    ins=[ar2i.opt()],
                    outs=[ar2o.opt()],
                )
                nc.sync.dma_start(out=st2g[:], in_=ar2o[:])

                # BN2 coefficients, vectorized over the 4 chunks
                mean4, ex24, m24 = co2[:, 0:4], co2[:, 4:8], co2[:, 8:12]
                var4, sv4, rsq4 = co2[:, 12:16], co2[:, 16:20], co2[:, 20:24]
                a2, b2 = co2[:, 24:28], co2[:, 28:32]
                nc.scalar.mul(mean4, st2g[:, 0:4], inv_n)
                nc.scalar.mul(ex24, st2g[:, 4:8], inv_n)
                nc.scalar.square(m24, mean4)
                nc.vector.scalar_tensor_tensor(
                    out=var4, in0=ex24, scalar=EPS, in1=m24,
                    op0=ALU.add, op1=ALU.subtract,
                )
                nc.scalar.activation(sv4, var4, AF.Sqrt)
                nc.vector.reciprocal(rsq4, sv4)
                nc.vector.tensor_mul(a2, g2b2[:, 0:4], rsq4)
                nc.vector.tensor_mul(m24, mean4, a2)      # reuse m24 as scratch
                nc.vector.tensor_sub(b2, g2b2[:, 4:8], m24)

                # BN2 apply + ReLU + DMA out; alternate ACT/DVE for balance
                for cc in range(NC2):
                    a_c = co2[:, 24 + cc : 25 + cc]
                    b_c = co2[:, 28 + cc : 29 + cc]
                    for t in range(NT):
                        ob = R.tile([128, 512], F32, tag="ob", name=f"ob_{cc}_{t}")
                        src = h2[cc][:, 512 * t : 512 * (t + 1)]
                        if (cc * NT + t) % 2 == 0:
                            nc.scalar.activation(
                                out=ob[:], in_=src, func=AF.Relu,
                                bias=b_c, scale=a_c,
                            )
                        else:
                            nc.vector.tensor_scalar(
                                out=ob[:], in0=src,
                                scalar1=a_c, scalar2=b_c,
                                op0=ALU.mult, op1=ALU.add,
                            )
                            nc.vector.tensor_scalar_max(ob[:], ob[:], 0.0)
                        nc.sync.dma_start(
                            out=out.ap()[cc * 128 : (cc + 1) * 128,
                                          512 * t : 512 * (t + 1)],
                            in_=ob[:],
                        )
    nc.compile()
    return nc


_CACHE = {}


def _get_nc():
    if "nc" not in _CACHE:
        nc = bacc.Bacc("TRN2", target_bir_lowering=False, debug=False,
                       num_devices=N_CORES)
        _CACHE["nc"] = _emit(nc)
    return _CACHE["nc"]


def _prep_inputs(x, w1, gamma1, bnb1, beta, w2, gamma2, bnb2):
    np_dt = np.float32
    if CONV_BF16:
        import ml_dtypes
        np_dt = ml_dtypes.bfloat16

    x = np.asarray(x, np.float32)
    xp = np.zeros((B, CIN, HP, WP), np.float32)
    xp[:, :, 1 : 1 + H, 1 : 1 + W] = x
    xp = xp.reshape(B, CIN, PPAD).astype(np_dt)

    w1t = (
        np.asarray(w1, np.float32)
        .transpose(1, 2, 3, 0)                     # [cin, ky, kx, cout]
        .reshape(NC1, 128, 9, 128)                 # [c, p, o, m]
        .transpose(1, 0, 2, 3)                     # [p, c, o, m]
        .reshape(128, NC1 * 9 * 128)
        .astype(np_dt)
    )
    w2t = (
        np.asarray(w2, np.float32)
        .transpose(1, 2, 3, 0)                     # [cin=128, ky, kx, cout=512]
        .reshape(128, 9, NC2, 128)                 # [p, o, cc, m]
        .reshape(128, 9 * NC2 * 128)
        .astype(np_dt)
    )
    g1b1 = np.stack(
        [np.asarray(gamma1, np.float32), np.asarray(bnb1, np.float32)], axis=1
    )
    g2b2 = np.concatenate(
        [
            np.asarray(gamma2, np.float32).reshape(NC2, 128).T,
            np.asarray(bnb2, np.float32).reshape(NC2, 128).T,
        ],
        axis=1,
    )
    betav = np.full((128, 1), np.asarray(beta, np.float32)[0], np.float32)
    ident = np.eye(128, dtype=np.float32)

    shared = {
        "w1t": np.ascontiguousarray(w1t),
        "w2t": np.ascontiguousarray(w2t),
        "g1b1": np.ascontiguousarray(g1b1),
        "g2b2": np.ascontiguousarray(g2b2),
        "betav": betav,
        "ident": ident,
    }
    return [dict(shared, xp=np.ascontiguousarray(xp[b])) for b in range(B)]


def kernel_with_results(inputs, trace=False):
    in_maps = _prep_inputs(**inputs)
    nc = _get_nc()
    res = bass_utils.run_bass_kernel_spmd(
        nc, in_maps, core_ids=list(range(N_CORES)), trace=trace
    )
    outs = np.stack([res.results[b]["out"] for b in range(B)])
    return outs.reshape(B, CIN, H, W).astype(np.float32), res


def kernel(**inputs):
    out, _ = kernel_with_results(inputs, trace=False)
    return out


# revision 12
# speedup vs baseline: 2.2214x; 2.2214x over previous
"""Trainium2 Bass kernel for nn_DAHead_Channel (conv3x3+BN+ReLU -> channel attention -> conv3x3+BN+ReLU).

Data-parallel over batch across 8 NeuronCores (1 batch element each).
Training-mode BatchNorm needs global (N,H,W) stats; conv1 uses one tiny
AllReduce, conv2 fires one AllReduce per output-channel chunk so the collective
latency hides under the next chunk's matmuls.

Convs run on the TensorEngine as 9 shifted matmuls over a zero-padded input in
float32r (full fp32 storage, 1 cycle/row PE streaming — ~4x faster than fp32
mode at ~1e-4 matmul accuracy). Attention Gram matrix stays fp32.

Hardcoded problem shape: x [8,512,64,64] f32, w1 [128,512,3,3], w2 [512,128,3,3].
"""

import numpy as np

import concourse.bacc as bacc
import concourse.mybir as mybir
import concourse.tile as tile
from concourse import bass_utils

N_CORES = 8
B, CIN, H, W = 8, 512, 64, 64
HP, WP = H + 2, W + 2          # zero-padded spatial dims
PIX = H * W                    # 4096
PPAD = HP * WP                 # 4356
NT = 8                         # HW tiles of 512 pixels (8 rows of 64)
NC1 = CIN // 128               # 4 input-channel chunks for conv1
NC2 = CIN // 128               # 4 output-channel chunks for conv2
NK = PIX // 128                # 32 transpose chunks for attention
EPS = 1e-5
NHW = B * PIX                  # BN count = 32768
F32 = mybir.dt.float32
F32R = mybir.dt.float32r


def _emit(nc):
    xin = nc.dram_tensor("xp", [CIN, PPAD], F32R, kind="ExternalInput")
    w1in = nc.dram_tensor("w1t", [128, NC1 * 9 * 128], F32R, kind="ExternalInput")
    w2in = nc.dram_tensor("w2t", [128, 9 * NC2 * 128], F32R, kind="ExternalInput")
    g1in = nc.dram_tensor("g1b1", [128, 2], F32, kind="ExternalInput")
    g2in = nc.dram_tensor("g2b2", [128, 8], F32, kind="ExternalInput")
    btin = nc.dram_tensor("betav", [128, 1], F32, kind="ExternalInput")
    idin = nc.dram_tensor("ident", [128, 128], F32R, kind="ExternalInput")
    out = nc.dram_tensor("out", [CIN, PIX], F32, kind="ExternalOutput")

    AF = mybir.ActivationFunctionType
    ALU = mybir.AluOpType
    AX = mybir.AxisListType
    inv_n = 1.0 / NHW

    with tile.TileContext(nc) as tc:
        with (
            tc.tile_pool(name="persist", bufs=1) as P,
            tc.tile_pool(name="rot", bufs=4) as R,
            tc.tile_pool(name="ps", bufs=8, space="PSUM") as PS,
            tc.tile_pool(name="dram", bufs=1, space="DRAM") as DR,
        ):
            # ---------------- persistent SBUF ----------------
            w2sb = P.tile([128, 9 * NC2 * 128], F32R, tag="w2")
            h1 = P.tile([128, PIX], F32, tag="h1")
            xf = P.tile([128, PIX], F32R, tag="xf")
            c2in = P.tile([128, PPAD], F32R, tag="c2in")
            ident = P.tile([128, 128], F32R, tag="ident")
            g1b1 = P.tile([128, 2], F32, tag="g1b1")
            g2b2 = P.tile([128, 8], F32, tag="g2b2")
            betav = P.tile([128, 1], F32, tag="betav")
            s1 = P.tile([128, NT], F32, tag="s1")
            q1 = P.tile([128, NT], F32, tag="q1")
            st1 = P.tile([128, 2], F32, tag="st1")
            st1g = P.tile([128, 2], F32, tag="st1g")
            s2 = P.tile([128, NC2 * NT], F32, tag="s2")
            q2 = P.tile([128, NC2 * NT], F32, tag="q2")
            st2 = P.tile([128, 8], F32, tag="st2")     # per-chunk [sum, sumsq]
            st2g = P.tile([128, 8], F32, tag="st2g")
            co1 = P.tile([128, 10], F32, tag="co1")
            co2 = P.tile([128, 32], F32, tag="co2")    # 8 cols per chunk
            att_e = P.tile([128, 128], F32, tag="att_e")
            attw = P.tile([128, 128], F32R, tag="attw")
            attwT = P.tile([128, 128], F32R, tag="attwT")
            asml = P.tile([128, 4], F32, tag="asml")

            nc.sync.dma_start(out=ident[:], in_=idin.ap()[:])
            nc.sync.dma_start(out=g1b1[:], in_=g1in.ap()[:])
            nc.sync.dma_start(out=g2b2[:], in_=g2in.ap()[:])
            nc.sync.dma_start(out=betav[:], in_=btin.ap()[:])
            nc.vector.memset(c2in[:].bitcast(F32), 0.0)

            ar1i = DR.tile([128, 2], F32, tag="ar1i")
            ar1o = DR.tile([128, 2], F32, tag="ar1o")
            ar2i = [DR.tile([128, 2], F32, tag=f"ar2i{c}", name=f"ar2i{c}")
                    for c in range(NC2)]
            ar2o = [DR.tile([128, 2], F32, tag=f"ar2o{c}", name=f"ar2o{c}")
                    for c in range(NC2)]

            # ---------------- phase 1: conv1 + attention ----------------
            with tc.tile_pool(name="phase1", bufs=1) as P1:
                w1sb = P1.tile([128, NC1 * 9 * 128], F32R, tag="w1")
                nc.sync.dma_start(out=w1sb[:], in_=w1in.ap()[:])
                xsb = []
                for c in range(NC1):
                    t = P1.tile([128, PPAD], F32R, tag=f"x{c}", name=f"xsb{c}")
                    nc.sync.dma_start(
                        out=t[:], in_=xin.ap()[c * 128 : (c + 1) * 128, :]
                    )
                    xsb.append(t)
                nc.sync.dma_start(out=w2sb[:], in_=w2in.ap()[:])

                ps1 = [
                    PS.tile([128, 512], F32, tag="ps", name=f"ps1_{t}")
                    for t in range(NT)
                ]
                for c in range(NC1):
                    xv = xsb[c][:].rearrange("p (h w) -> p h w", w=WP)
                    for o in range(9):
                        ky, kx = o // 3, o % 3
                        lhsT = w1sb[:, (c * 9 + o) * 128 : (c * 9 + o + 1) * 128]
                        for t in range(NT):
                            nc.tensor.matmul(
                                ps1[t][:],
                                lhsT,
                                xv[:, 8 * t + ky : 8 * t + ky + 8, kx : kx + W],
                                start=(c == 0 and o == 0),
                                stop=(c == NC1 - 1 and o == 8),
                                skip_group_check=True,
                            )
                # drain: copy to h1 (+rowsum on ACT), square (+rowsumsq on DVE)
                for t in range(NT):
                    sc = R.tile([128, 512], F32, tag="scr", name=f"scr1_{t}")
                    nc.scalar.activation(
                        out=h1[:, 512 * t : 512 * (t + 1)],
                        in_=ps1[t][:],
                        func=AF.Copy,
                        accum_out=s1[:, t : t + 1],
                    )
                    nc.vector.scalar_tensor_tensor(
                        out=sc[:],
                        in0=ps1[t][:],
                        scalar=1.0,
                        in1=h1[:, 512 * t : 512 * (t + 1)],
                        op0=ALU.mult,
                        op1=ALU.mult,
                        accum_out=q1[:, t : t + 1],
                    )

                # local stats -> AllReduce
                nc.vector.reduce_sum(st1[:, 0:1], s1[:], axis=AX.X)
                nc.vector.reduce_sum(st1[:, 1:2], q1[:], axis=AX.X)
                nc.sync.dma_start(out=ar1i[:], in_=st1[:])
                nc.gpsimd.collective_compute(
                    "AllReduce",
                    ALU.add,
                    replica_groups=[list(range(N_CORES))],
                    ins=[ar1i.opt()],
                    outs=[ar1o.opt()],
                )
                nc.sync.dma_start(out=st1g[:], in_=ar1o[:])

                # BN1 coefficients: a = gamma*rsqrt(var+eps), b = beta - mean*a
                mean, ex2, m2, var = co1[:, 0:1], co1[:, 1:2], co1[:, 2:3], co1[:, 3:4]
                sv, rsq, a1 = co1[:, 4:5], co1[:, 5:6], co1[:, 6:7]
                ma, b1 = co1[:, 7:8], co1[:, 8:9]
                nc.scalar.mul(mean, st1g[:, 0:1], inv_n)
                nc.scalar.mul(ex2, st1g[:, 1:2], inv_n)
                nc.scalar.square(m2, mean)
                nc.vector.scalar_tensor_tensor(
                    out=var, in0=ex2, scalar=EPS, in1=m2,
                    op0=ALU.add, op1=ALU.subtract,
                )
                nc.scalar.activation(sv, var, AF.Sqrt)
                nc.vector.reciprocal(rsq, sv)
                nc.vector.tensor_mul(a1, g1b1[:, 0:1], rsq)
                nc.vector.tensor_mul(ma, mean, a1)
                nc.vector.tensor_sub(b1, g1b1[:, 1:2], ma)

                # BN1 apply + ReLU, then transpose chunks + Gram accumulation.
                att = PS.tile([128, 512], F32, tag="ps", name="att")
                for t in range(NT):
                    nc.scalar.activation(
                        out=xf[:, 512 * t : 512 * (t + 1)],
                        in_=h1[:, 512 * t : 512 * (t + 1)],
                        func=AF.Relu,
                        bias=b1,
                        scale=a1,
                    )
                    for k in range(4 * t, 4 * t + 4):
                        pst = PS.tile([128, 512], F32R, tag="ps", name=f"pst{k}")
                        xfT = P1.tile([128, 128], F32, tag="xfT", bufs=3,
                                      name=f"xfT{k}")
                        nc.tensor.transpose(
                            pst[:, 0:128], xf[:, 128 * k : 128 * (k + 1)], ident[:]
                        )
                        nc.vector.tensor_copy(xfT[:], pst[:, 0:128].bitcast(F32))
                        nc.tensor.matmul(
                            att[:, 0:128],
                            xfT[:],
                            xfT[:],
                            start=(k == 0),
                            stop=(k == NK - 1),
                            skip_group_check=True,
                        )

                # softmax of (rowmax - att) over rows == exp(rowmin - att)/sum
                amin, asum, arcp = asml[:, 0:1], asml[:, 1:2], asml[:, 2:3]
                nc.vector.tensor_reduce(
                    out=amin, in_=att[:, 0:128], op=ALU.min, axis=AX.X
                )
                nc.scalar.activation(
                    out=att_e[:],
                    in_=att[:, 0:128],
                    func=AF.Exp,
                    bias=amin,
                    scale=-1.0,
                    accum_out=asum,
                )
                nc.vector.reciprocal(arcp, asum)
                nc.vector.tensor_scalar_mul(attw[:], att_e[:], arcp)
                pat = PS.tile([128, 512], F32R, tag="ps", name="pat")
                nc.tensor.transpose(pat[:, 0:128], attw[:], ident[:])
                nc.vector.tensor_copy(attwT[:], pat[:, 0:128])

                # out = attw @ xf ; c2in = beta*out + xf (into padded interior)
                c2v = c2in[:].rearrange("p (h w) -> p h w", w=WP)
                for t in range(NT):
                    po = PS.tile([128, 512], F32, tag="ps", name=f"po{t}")
                    nc.tensor.matmul(
                        po[:],
                        attwT[:],
                        xf[:, 512 * t : 512 * (t + 1)],
                        start=True, stop=True, skip_group_check=True,
                    )
                    nc.vector.scalar_tensor_tensor(
                        out=c2v[:, 1 + 8 * t : 9 + 8 * t, 1 : 1 + W],
                        in0=po[:],
                        scalar=betav[:],
                        in1=xf[:, 512 * t : 512 * (t + 1)].bitcast(F32),
                        op0=ALU.mult,
                        op1=ALU.add,
                    )

            # ---------------- phase 2: conv2, pipelined per chunk ----------------
            with tc.tile_pool(name="phase2", bufs=1) as P2:
                c2vv = c2in[:].rearrange("p (h w) -> p h w", w=WP)
                for cc in range(NC2):
                    h2 = P2.tile([128, PIX], F32, tag="h2", bufs=3, name=f"h2_{cc}")
                    ps2 = [
                        PS.tile([128, 512], F32, tag="ps", name=f"ps2_{cc}_{t}")
                        for t in range(NT)
                    ]
                    for o in range(9):
                        ky, kx = o // 3, o % 3
                        lhsT = w2sb[:, (o * NC2 + cc) * 128 : (o * NC2 + cc + 1) * 128]
                        for t in range(NT):
                            nc.tensor.matmul(
                                ps2[t][:],
                                lhsT,
                                c2vv[:, 8 * t + ky : 8 * t + ky + 8,
                                     kx : kx + W],
                                start=(o == 0),
                                stop=(o == 8),
                                skip_group_check=True,
                            )
                    for t in range(NT):
                        sc = R.tile([128, 512], F32, tag="scr", name=f"scr2_{cc}_{t}")
                        nc.scalar.activation(
                            out=h2[:, 512 * t : 512 * (t + 1)],
                            in_=ps2[t][:],
                            func=AF.Copy,
                            accum_out=s2[:, cc * NT + t : cc * NT + t + 1],
                        )
                        nc.vector.scalar_tensor_tensor(
                            out=sc[:],
                            in0=ps2[t][:],
                            scalar=1.0,
                            in1=h2[:, 512 * t : 512 * (t + 1)],
                            op0=ALU.mult,
                            op1=ALU.mult,
                            accum_out=q2[:, cc * NT + t : cc * NT + t + 1],
                        )

                    # per-chunk stats AllReduce (overlaps next chunk's matmuls)
                    nc.vector.reduce_sum(
                        st2[:, 2 * cc : 2 * cc + 1],
                        s2[:, cc * NT : (cc + 1) * NT], axis=AX.X,
                    )
                    nc.vector.reduce_sum(
                        st2[:, 2 * cc + 1 : 2 * cc + 2],
                        q2[:, cc * NT : (cc + 1) * NT], axis=AX.X,
                    )
                    nc.sync.dma_start(
                        out=ar2i[cc][:], in_=st2[:, 2 * cc : 2 * cc + 2]
                    )
                    nc.gpsimd.collective_compute(
                        "AllReduce",
                        ALU.add,
                        replica_groups=[list(range(N_CORES))],
                        ins=[ar2i[cc].opt()],
                        outs=[ar2o[cc].opt()],
                    )
                    nc.sync.dma_start(
                        out=st2g[:, 2 * cc : 2 * cc + 2], in_=ar2o[cc][:]
                    )

                    # BN2 coefficients for this chunk
                    base = 8 * cc
                    mean = co2[:, base + 0 : base + 1]
                    ex2 = co2[:, base + 1 : base + 2]
                    m2 = co2[:, base + 2 : base + 3]
                    var = co2[:, base + 3 : base + 4]
                    sv = co2[:, base + 4 : base + 5]
                    rsq = co2[:, base + 5 : base + 6]
                    a2 = co2[:, base + 6 : base + 7]
                    b2 = co2[:, base + 7 : base + 8]
                    nc.scalar.mul(mean, st2g[:, 2 * cc : 2 * cc + 1], inv_n)
                    nc.scalar.mul(ex2, st2g[:, 2 * cc + 1 : 2 * cc + 2], inv_n)
                    nc.scalar.square(m2, mean)
                    nc.vector.scalar_tensor_tensor(
                        out=var, in0=ex2, scalar=EPS, in1=m2,
                        op0=ALU.add, op1=ALU.subtract,
                    )
                    nc.scalar.activation(sv, var, AF.Sqrt)
                    nc.vector.reciprocal(rsq, sv)
                    nc.vector.tensor_mul(a2, g2b2[:, cc : cc + 1], rsq)
                    nc.vector.tensor_mul(m2, mean, a2)          # m2 as scratch
                    nc.vector.tensor_sub(b2, g2b2[:, 4 + cc : 5 + cc], m2)

                    # BN2 apply + ReLU + DMA out; alternate ACT/DVE
                    for t in range(NT):
                        ob = R.tile([128, 512], F32, tag="ob", name=f"ob_{cc}_{t}")
                        src = h2[:, 512 * t : 512 * (t + 1)]
                        if t % 2 == 0:
                            nc.scalar.activation(
                                out=ob[:], in_=src, func=AF.Relu,
                                bias=b2, scale=a2,
                            )
                        else:
                            nc.vector.tensor_scalar(
                                out=ob[:], in0=src,
                                scalar1=a2, scalar2=b2,
                                op0=ALU.mult, op1=ALU.add,
                            )
                            nc.vector.tensor_scalar_max(ob[:], ob[:], 0.0)
                        nc.sync.dma_start(
                            out=out.ap()[cc * 128 : (cc + 1) * 128,
                                          512 * t : 512 * (t + 1)],
                            in_=ob[:],
                        )
    nc.compile()
    return nc


_CACHE = {}


def _get_nc():
    if "nc" not in _CACHE:
        nc = bacc.Bacc("TRN2", target_bir_lowering=False, debug=False,
                       num_devices=N_CORES)
        _CACHE["nc"] = _emit(nc)
    return _CACHE["nc"]


def _prep_inputs(x, w1, gamma1, bnb1, beta, w2, gamma2, bnb2):
    x = np.asarray(x, np.float32)
    xp = np.zeros((B, CIN, HP, WP), np.float32)
    xp[:, :, 1 : 1 + H, 1 : 1 + W] = x
    xp = xp.reshape(B, CIN, PPAD)

    w1t = (
        np.asarray(w1, np.float32)
        .transpose(1, 2, 3, 0)                     # [cin, ky, kx, cout]
        .reshape(NC1, 128, 9, 128)                 # [c, p, o, m]
        .transpose(1, 0, 2, 3)                     # [p, c, o, m]
        .reshape(128, NC1 * 9 * 128)
    )
    w2t = (
        np.asarray(w2, np.float32)
        .transpose(1, 2, 3, 0)                     # [cin=128, ky, kx, cout=512]
        .reshape(128, 9, NC2, 128)                 # [p, o, cc, m]
        .reshape(128, 9 * NC2 * 128)
    )
    g1b1 = np.stack(
        [np.asarray(gamma1, np.float32), np.asarray(bnb1, np.float32)], axis=1
    )
    g2b2 = np.concatenate(
        [
            np.asarray(gamma2, np.float32).reshape(NC2, 128).T,
            np.asarray(bnb2, np.float32).reshape(NC2, 128).T,
        ],
        axis=1,
    )
    betav = np.full((128, 1), np.asarray(beta, np.float32)[0], np.float32)
    ident = np.eye(128, dtype=np.float32)

    shared = {
        "w1t": np.ascontiguousarray(w1t),
        "w2t": np.ascontiguousarray(w2t),
        "g1b1": np.ascontiguousarray(g1b1),
        "g2b2": np.ascontiguousarray(g2b2),
        "betav": betav,
        "ident": ident,
    }
    return [dict(shared, xp=np.ascontiguousarray(xp[b])) for b in range(B)]


def kernel_with_results(inputs, trace=False):
    in_maps = _prep_inputs(**inputs)
    nc = _get_nc()
    res = bass_utils.run_bass_kernel_spmd(
        nc, in_maps, core_ids=list(range(N_CORES)), trace=trace
    )
    outs = np.stack([res.results[b]["out"] for b in range(B)])
    return outs.reshape(B, CIN, H, W).astype(np.float32), res


def kernel(**inputs):
    out, _ = kernel_with_results(inputs, trace=False)
    return out


# revision 20
# speedup vs baseline: 3.0490x; 1.3725x over previous
"""Trainium2 Bass kernel for nn_DAHead_Channel (conv3x3+BN+ReLU -> channel attention -> conv3x3+BN+ReLU).

Data-parallel over batch across 8 NeuronCores (1 batch element each).
Training-mode BatchNorm needs global (N,H,W) stats; conv1 uses one tiny
AllReduce, conv2 fires one AllReduce per output-channel chunk so the collective
latency hides under the next chunk's matmuls.

Convs run on the TensorEngine as 9 shifted matmuls over a zero-padded input in
float32r (full fp32 storage, 1 cycle/row PE streaming — ~4x faster than fp32
mode at ~1e-4 matmul accuracy). Attention Gram matrix stays fp32.

Hardcoded problem shape: x [8,512,64,64] f32, w1 [128,512,3,3], w2 [512,128,3,3].
"""

import numpy as np

import concourse.bacc as bacc
import concourse.mybir as mybir
import concourse.tile as tile
from concourse import bass_utils

N_CORES = 8
B, CIN, H, W = 8, 512, 64, 64
HP, WP = H + 2, W + 2          # zero-padded spatial dims
PIX = H * W                    # 4096
PPAD = HP * WP                 # 4356
NT = 8                         # HW tiles of 512 pixels (8 rows of 64)
NC1 = CIN // 128               # 4 input-channel chunks for conv1
NC2 = CIN // 128               # 4 output-channel chunks for conv2
NK = PIX // 128                # 32 transpose chunks for attention
EPS = 1e-5
NHW = B * PIX                  # BN count = 32768
F32 = mybir.dt.float32
F32R = mybir.dt.float32r


def _emit(nc):
    xin = nc.dram_tensor("xp", [CIN, PPAD], F32R, kind="ExternalInput")
    w1in = nc.dram_tensor("w1t", [128, NC1 * 9 * 128], F32R, kind="ExternalInput")
    w2in = nc.dram_tensor("w2t", [128, 9 * NC2 * 128], F32R, kind="ExternalInput")
    g1in = nc.dram_tensor("g1b1", [128, 2], F32, kind="ExternalInput")
    g2in = nc.dram_tensor("g2b2", [128, 8], F32, kind="ExternalInput")
    btin = nc.dram_tensor("betav", [128, 1], F32, kind="ExternalInput")
    idin = nc.dram_tensor("ident", [128, 128], F32R, kind="ExternalInput")
    out = nc.dram_tensor("out", [CIN, PIX], F32, kind="ExternalOutput")

    AF = mybir.ActivationFunctionType
    ALU = mybir.AluOpType
    AX = mybir.AxisListType
    inv_n = 1.0 / NHW

    with tile.TileContext(nc) as tc:
        with (
            tc.tile_pool(name="persist", bufs=1) as P,
            tc.tile_pool(name="rot", bufs=4) as R,
            tc.tile_pool(name="ps", bufs=8, space="PSUM") as PS,
            tc.tile_pool(name="dram", bufs=1, space="DRAM") as DR,
        ):
            # ---------------- persistent SBUF ----------------
            w2sb = P.tile([128, 9 * NC2 * 128], F32R, tag="w2")
            h1 = P.tile([128, PIX], F32, tag="h1")
            xf = P.tile([128, PIX], F32R, tag="xf")
            c2in = P.tile([128, PPAD], F32R, tag="c2in")
            ident = P.tile([128, 128], F32R, tag="ident")
            g1b1 = P.tile([128, 2], F32, tag="g1b1")
            g2b2 = P.tile([128, 8], F32, tag="g2b2")
            betav = P.tile([128, 1], F32, tag="betav")
            s1 = P.tile([128, NT], F32, tag="s1")
            q1 = P.tile([128, NT], F32, tag="q1")
            st1 = P.tile([128, 2], F32, tag="st1")
            st1g = P.tile([128, 2], F32, tag="st1g")
            s2 = P.tile([128, NC2 * NT], F32, tag="s2")
            q2 = P.tile([128, NC2 * NT], F32, tag="q2")
            st2 = P.tile([128, 8], F32, tag="st2")     # per-chunk [sum, sumsq]
            st2g = P.tile([128, 8], F32, tag="st2g")
            co1 = P.tile([128, 10], F32, tag="co1")
            co2 = P.tile([128, 32], F32, tag="co2")    # 8 cols per chunk
            att_e = P.tile([128, 128], F32, tag="att_e")
            attw = P.tile([128, 128], F32R, tag="attw")
            attwT = P.tile([128, 128], F32R, tag="attwT")
            asml = P.tile([128, 4], F32, tag="asml")

            nc.vector.memset(c2in[:].bitcast(F32), 0.0)

            ar0i = DR.tile([128, 1], F32, tag="ar0i")
            ar0o = DR.tile([128, 1], F32, tag="ar0o", addr_space="Shared")
            zsc = P.tile([128, 1], F32, tag="zsc")
            zsc2 = P.tile([128, 1], F32, tag="zsc2")

            ar1i = DR.tile([128, 2], F32, tag="ar1i")
            ar1o = DR.tile([128, 2], F32, tag="ar1o", addr_space="Shared")
            ar2i = [DR.tile([128, 2], F32, tag=f"ar2i{c}", name=f"ar2i{c}")
                    for c in range(NC2)]
            ar2o = [DR.tile([128, 2], F32, tag=f"ar2o{c}", name=f"ar2o{c}",
                            addr_space="Shared")
                    for c in range(NC2)]

            # ---------------- phase 1: conv1 + attention ----------------
            with tc.tile_pool(name="phase1", bufs=1) as P1:
                w1sb = P1.tile([128, NC1 * 9 * 128], F32R, tag="w1")
                xsb = [
                    P1.tile([128, PPAD], F32R, tag=f"x{c}", name=f"xsb{c}")
                    for c in range(NC1)
                ]
                # interleave weight-chunk and x-chunk DMAs (split for fast start)
                for c in range(NC1):
                    nc.sync.dma_start(
                        out=w1sb[:, c * 1152 : (c + 1) * 1152],
                        in_=w1in.ap()[:, c * 1152 : (c + 1) * 1152],
                    )
                    half = PPAD // 2
                    nc.sync.dma_start(
                        out=xsb[c][:, 0:half],
                        in_=xin.ap()[c * 128 : (c + 1) * 128, 0:half],
                    )
                    nc.sync.dma_start(
                        out=xsb[c][:, half:PPAD],
                        in_=xin.ap()[c * 128 : (c + 1) * 128, half:PPAD],
                    )
                # warmup: collective staging + ACT tables (off critical path)
                nc.vector.memset(zsc[:], 0.0)
                nc.sync.dma_start(out=ar0i[:], in_=zsc[:])
                nc.scalar.activation(zsc2[:], zsc[:], AF.Sqrt)
                nc.scalar.activation(zsc2[:], zsc[:], AF.Exp)
                nc.gpsimd.collective_compute(
                    "AllReduce",
                    mybir.AluOpType.add,
                    replica_groups=[list(range(N_CORES))],
                    ins=[ar0i.opt()],
                    outs=[ar0o.opt()],
                )
                # small params + conv2 weights after the conv1-critical loads
                nc.sync.dma_start(out=ident[:], in_=idin.ap()[:])
                nc.sync.dma_start(out=g1b1[:], in_=g1in.ap()[:])
                nc.sync.dma_start(out=g2b2[:], in_=g2in.ap()[:])
                nc.sync.dma_start(out=betav[:], in_=btin.ap()[:])
                nc.sync.dma_start(out=w2sb[:], in_=w2in.ap()[:])

                ps1 = [
                    PS.tile([128, 512], F32, tag="ps", name=f"ps1_{t}")
                    for t in range(NT)
                ]
                for c in range(NC1):
                    xv = xsb[c][:].rearrange("p (h w) -> p h w", w=WP)
                    for t in range(NT):
                        for o in range(9):
                            ky, kx = o // 3, o % 3
                            lhsT = w1sb[:, (c * 9 + o) * 128 : (c * 9 + o + 1) * 128]
                            nc.tensor.matmul(
                                ps1[t][:],
                                lhsT,
                                xv[:, 8 * t + ky : 8 * t + ky + 8, kx : kx + W],
                                start=(c == 0 and o == 0),
                                stop=(c == NC1 - 1 and o == 8),
                                skip_group_check=True,
                            )
                        if c == NC1 - 1:
                            # drain as soon as this tile's accumulation closes
                            sc = R.tile([128, 512], F32, tag="scr", name=f"scr1_{t}")
                            nc.scalar.activation(
                                out=h1[:, 512 * t : 512 * (t + 1)],
                                in_=ps1[t][:],
                                func=AF.Copy,
                                accum_out=s1[:, t : t + 1],
                            )
                            nc.vector.scalar_tensor_tensor(
                                out=sc[:],
                                in0=ps1[t][:],
                                scalar=1.0,
                                in1=h1[:, 512 * t : 512 * (t + 1)],
                                op0=ALU.mult,
                                op1=ALU.mult,
                                accum_out=q1[:, t : t + 1],
                            )

                # local stats -> AllReduce
                nc.vector.reduce_sum(st1[:, 0:1], s1[:], axis=AX.X)
                nc.vector.reduce_sum(st1[:, 1:2], q1[:], axis=AX.X)
                nc.sync.dma_start(out=ar1i[:], in_=st1[:])
                nc.gpsimd.collective_compute(
                    "AllReduce",
                    ALU.add,
                    replica_groups=[list(range(N_CORES))],
                    ins=[ar1i.opt()],
                    outs=[ar1o.opt()],
                )
                nc.gpsimd.dma_start(out=st1g[:], in_=ar1o[:])

                # BN1 coefficients: a = gamma*rsqrt(var+eps), b = beta - mean*a
                mean, ex2, m2, var = co1[:, 0:1], co1[:, 1:2], co1[:, 2:3], co1[:, 3:4]
                sv, rsq, a1 = co1[:, 4:5], co1[:, 5:6], co1[:, 6:7]
                ma, b1 = co1[:, 7:8], co1[:, 8:9]
                nc.scalar.mul(mean, st1g[:, 0:1], inv_n)
                nc.scalar.mul(ex2, st1g[:, 1:2], inv_n)
                nc.scalar.square(m2, mean)
                nc.vector.scalar_tensor_tensor(
                    out=var, in0=ex2, scalar=EPS, in1=m2,
                    op0=ALU.add, op1=ALU.subtract,
                )
                nc.scalar.activation(sv, var, AF.Sqrt)
                nc.vector.reciprocal(rsq, sv)
                nc.vector.tensor_mul(a1, g1b1[:, 0:1], rsq)
                nc.vector.tensor_mul(ma, mean, a1)
                nc.vector.tensor_sub(b1, g1b1[:, 1:2], ma)

                # BN1 apply + ReLU, then transpose chunks + Gram accumulation.
                att = PS.tile([128, 512], F32, tag="ps", name="att")
                for t in range(NT):
                    nc.scalar.activation(
                        out=xf[:, 512 * t : 512 * (t + 1)],
                        in_=h1[:, 512 * t : 512 * (t + 1)],
                        func=AF.Relu,
                        bias=b1,
                        scale=a1,
                    )
                    for k in range(4 * t, 4 * t + 4):
                        pst = PS.tile([128, 512], F32R, tag="ps", name=f"pst{k}")
                        xfT = P1.tile([128, 128], F32, tag="xfT", bufs=3,
                                      name=f"xfT{k}")
                        nc.tensor.transpose(
                            pst[:, 0:128], xf[:, 128 * k : 128 * (k + 1)], ident[:]
                        )
                        nc.vector.tensor_copy(xfT[:], pst[:, 0:128].bitcast(F32))
                        nc.tensor.matmul(
                            att[:, 0:128],
                            xfT[:],
                            xfT[:],
                            start=(k == 0),
                            stop=(k == NK - 1),
                            skip_group_check=True,
                        )

                # softmax of (rowmax - att) over rows == exp(rowmin - att)/sum
                amin, asum, arcp = asml[:, 0:1], asml[:, 1:2], asml[:, 2:3]
                nc.vector.tensor_reduce(
                    out=amin, in_=att[:, 0:128], op=ALU.min, axis=AX.X
                )
                nc.scalar.activation(
                    out=att_e[:],
                    in_=att[:, 0:128],
                    func=AF.Exp,
                    bias=amin,
                    scale=-1.0,
                    accum_out=asum,
                )
                nc.vector.reciprocal(arcp, asum)
                nc.vector.tensor_scalar_mul(attw[:], att_e[:], arcp)
                pat = PS.tile([128, 512], F32R, tag="ps", name="pat")
                nc.tensor.transpose(pat[:, 0:128], attw[:], ident[:])
                nc.vector.tensor_copy(attwT[:], pat[:, 0:128])

                # out = attw @ xf ; c2in = beta*out + xf (into padded interior)
                c2v = c2in[:].rearrange("p (h w) -> p h w", w=WP)
                for t in range(NT):
                    po = PS.tile([128, 512], F32, tag="ps", name=f"po{t}")
                    nc.tensor.matmul(
                        po[:],
                        attwT[:],
                        xf[:, 512 * t : 512 * (t + 1)],
                        start=True, stop=True, skip_group_check=True,
                    )
                    nc.vector.scalar_tensor_tensor(
                        out=c2v[:, 1 + 8 * t : 9 + 8 * t, 1 : 1 + W],
                        in0=po[:],
                        scalar=betav[:],
                        in1=xf[:, 512 * t : 512 * (t + 1)].bitcast(F32),
                        op0=ALU.mult,
                        op1=ALU.add,
                    )

            # ---------------- phase 2: conv2, stats AR per chunk ----------------
            with tc.tile_pool(name="phase2", bufs=1) as P2:
                c2vv = c2in[:].rearrange("p (h w) -> p h w", w=WP)
                h2 = [
                    P2.tile([128, PIX], F32, tag=f"h2_{cc}", name=f"h2_{cc}")
                    for cc in range(NC2)
                ]
                def bn2_coefs(cc):
                    base = 8 * cc
                    mean = co2[:, base + 0 : base + 1]
                    ex2 = co2[:, base + 1 : base + 2]
                    m2 = co2[:, base + 2 : base + 3]
                    var = co2[:, base + 3 : base + 4]
                    sv = co2[:, base + 4 : base + 5]
                    rsq = co2[:, base + 5 : base + 6]
                    a2 = co2[:, base + 6 : base + 7]
                    b2 = co2[:, base + 7 : base + 8]
                    nc.scalar.mul(mean, st2g[:, 2 * cc : 2 * cc + 1], inv_n)
                    nc.scalar.mul(ex2, st2g[:, 2 * cc + 1 : 2 * cc + 2], inv_n)
                    nc.scalar.square(m2, mean)
                    nc.vector.scalar_tensor_tensor(
                        out=var, in0=ex2, scalar=EPS, in1=m2,
                        op0=ALU.add, op1=ALU.subtract,
                    )
                    nc.scalar.activation(sv, var, AF.Sqrt)
                    nc.vector.reciprocal(rsq, sv)
                    nc.vector.tensor_mul(a2, g2b2[:, cc : cc + 1], rsq)
                    nc.vector.tensor_mul(m2, mean, a2)
                    nc.vector.tensor_sub(b2, g2b2[:, 4 + cc : 5 + cc], m2)

                def bn2_apply(cc):
                    a_c = co2[:, 8 * cc + 6 : 8 * cc + 7]
                    b_c = co2[:, 8 * cc + 7 : 8 * cc + 8]
                    for t in range(NT):
                        ob = R.tile([128, 512], F32, tag="ob", name=f"ob_{cc}_{t}")
                        hsrc = h2[cc][:, 512 * t : 512 * (t + 1)]
                        if t % 2 == 0:
                            nc.scalar.activation(
                                out=ob[:], in_=hsrc, func=AF.Relu,
                                bias=b_c, scale=a_c,
                            )
                        else:
                            nc.vector.tensor_scalar(
                                out=ob[:], in0=hsrc,
                                scalar1=a_c, scalar2=b_c,
                                op0=ALU.mult, op1=ALU.add,
                            )
                            nc.vector.tensor_scalar_max(ob[:], ob[:], 0.0)
                        nc.sync.dma_start(
                            out=out.ap()[cc * 128 : (cc + 1) * 128,
                                          512 * t : 512 * (t + 1)],
                            in_=ob[:],
                        )

                for cc in range(NC2):
                    for t in range(NT):
                        ps2 = PS.tile([128, 512], F32, tag="ps",
                                      name=f"ps2_{cc}_{t}")
                        for o in range(9):
                            ky, kx = o // 3, o % 3
                            lhsT = w2sb[:, (o * NC2 + cc) * 128
                                        : (o * NC2 + cc + 1) * 128]
                            nc.tensor.matmul(
                                ps2[:],
                                lhsT,
                                c2vv[:, 8 * t + ky : 8 * t + ky + 8,
                                     kx : kx + W],
                                start=(o == 0),
                                stop=(o == 8),
                                skip_group_check=True,
                            )
                        sc = R.tile([128, 512], F32, tag="scr", name=f"scr2_{cc}_{t}")
                        nc.scalar.activation(
                            out=h2[cc][:, 512 * t : 512 * (t + 1)],
                            in_=ps2[:],
                            func=AF.Copy,
                            accum_out=s2[:, cc * NT + t : cc * NT + t + 1],
                        )
                        nc.vector.scalar_tensor_tensor(
                            out=sc[:],
                            in0=ps2[:],
                            scalar=1.0,
                            in1=h2[cc][:, 512 * t : 512 * (t + 1)],
                            op0=ALU.mult,
                            op1=ALU.mult,
                            accum_out=q2[:, cc * NT + t : cc * NT + t + 1],
                        )

                    # per-chunk stats AllReduce (overlaps remaining chunks)
                    nc.vector.reduce_sum(
                        st2[:, 2 * cc : 2 * cc + 1],
                        s2[:, cc * NT : (cc + 1) * NT], axis=AX.X,
                    )
                    nc.vector.reduce_sum(
                        st2[:, 2 * cc + 1 : 2 * cc + 2],
                        q2[:, cc * NT : (cc + 1) * NT], axis=AX.X,
                    )
                    nc.sync.dma_start(
                        out=ar2i[cc][:], in_=st2[:, 2 * cc : 2 * cc + 2]
                    )
                    nc.gpsimd.collective_compute(
                        "AllReduce",
                        ALU.add,
                        replica_groups=[list(range(N_CORES))],
                        ins=[ar2i[cc].opt()],
                        outs=[ar2o[cc].opt()],
                    )
                    nc.gpsimd.dma_start(
                        out=st2g[:, 2 * cc : 2 * cc + 2], in_=ar2o[cc][:]
                    )
                    if cc >= 2:
                        bn2_coefs(cc - 2)
                        bn2_apply(cc - 2)

                for cc in range(NC2 - 2, NC2):
                    bn2_coefs(cc)
                    bn2_apply(cc)
    nc.compile()
    return nc


_CACHE = {}


def _get_nc():
    if "nc" not in _CACHE:
        nc = bacc.Bacc("TRN2", target_bir_lowering=False, debug=False,
                       num_devices=N_CORES)
        _CACHE["nc"] = _emit(nc)
    return _CACHE["nc"]


def _prep_inputs(x, w1, gamma1, bnb1, beta, w2, gamma2, bnb2):
    x = np.asarray(x, np.float32)
    xp = np.zeros((B, CIN, HP, WP), np.float32)
    xp[:, :, 1 : 1 + H, 1 : 1 + W] = x
    xp = xp.reshape(B, CIN, PPAD)

    w1t = (
        np.asarray(w1, np.float32)
        .transpose(1, 2, 3, 0)                     # [cin, ky, kx, cout]
        .reshape(NC1, 128, 9, 128)                 # [c, p, o, m]
        .transpose(1, 0, 2, 3)                     # [p, c, o, m]
        .reshape(128, NC1 * 9 * 128)
    )
    w2t = (
        np.asarray(w2, np.float32)
        .transpose(1, 2, 3, 0)                     # [cin=128, ky, kx, cout=512]
        .reshape(128, 9, NC2, 128)                 # [p, o, cc, m]
        .reshape(128, 9 * NC2 * 128)
    )
    g1b1 = np.stack(
        [np.asarray(gamma1, np.float32), np.asarray(bnb1, np.float32)], axis=1
    )
    g2b2 = np.concatenate(
        [
            np.asarray(gamma2, np.float32).reshape(NC2, 128).T,
            np.asarray(bnb2, np.float32).reshape(NC2, 128).T,
        ],
        axis=1,
    )
    betav = np.full((128, 1), np.asarray(beta, np.float32)[0], np.float32)
    ident = np.eye(128, dtype=np.float32)

    shared = {
        "w1t": np.ascontiguousarray(w1t),
        "w2t": np.ascontiguousarray(w2t),
        "g1b1": np.ascontiguousarray(g1b1),
        "g2b2": np.ascontiguousarray(g2b2),
        "betav": betav,
        "ident": ident,
    }
    return [dict(shared, xp=np.ascontiguousarray(xp[b])) for b in range(B)]


def kernel_with_results(inputs, trace=False):
    in_maps = _prep_inputs(**inputs)
    nc = _get_nc()
    res = bass_utils.run_bass_kernel_spmd(
        nc, in_maps, core_ids=list(range(N_CORES)), trace=trace
    )
    outs = np.stack([res.results[b]["out"] for b in range(B)])
    return outs.reshape(B, CIN, H, W).astype(np.float32), res


def kernel(**inputs):
    out, _ = kernel_with_results(inputs, trace=False)
    return out
